# revision 1
# baseline (speedup 1.0000x reference)
"""GQA attention (dense_transformer) distributed over 8 TRN2 NeuronCores.

Sharding: batch (2) x head-groups (4). Core c = 4*b + g handles batch b,
q-heads 4g..4g+3 and kv-head g (GQA group local). Megatron-style:
 - QKV projection with column-sharded weights, x^T replicated per batch group
 - RoPE fused into the PSUM->SBUF eviction (host permutes wq/wk columns to
   [even dims; odd dims] per head so rotation is a partition-block affair);
   the final rotation combines run on gpsimd so the DVE keeps up
 - attention computed transposed (scoresT: k on partitions, q on free) so the
   AV matmul needs no transposes
 - causal: projection s-chunks are interleaved with attention rounds (round j
   only needs chunks <= j), so the per-round AllGathers fire early and hide
   entirely under later projection chunks
 - softmax denominators: exp tiles are pair/quad-reduced on the DVE (bf16),
   then a short ones-matmul accumulation over the quad tiles; the causal
   diagonal bias is added by the DVE directly into the score PSUM
 - after each q-chunk j, a 4-rank AllGather shares attnT[:, chunk j] (all 16
   heads) with the batch group; every core then runs the out-projection for
   chunk j against ITS OWN 512-column slice of wo (a per-core host input, so
   the graph stays rank-independent) while later work runs. Chunk 3's
   AllGather is split in head-pair halves so its projection can start while
   the second half is still in flight.
 - output is written bf16 as (S, 512 cols per core); host concatenates
   column blocks and upcasts to f32.

All matmul operands are bf16 (fp32 PSUM accumulation); softmax runs in fp32
on the scalar engine with a constant shift folded into the exp bias.
"""

import os
import numpy as np

B = 2
S = 2048
DIM = 2048
NH = 16
NKV = 4
HD = 128
NCORES = 8
QH = NH // NKV  # q heads per core (= per kv group)
SC = 512  # q-chunk / s-chunk size
NSC = S // SC  # 4
NKT = S // HD  # 16 k-tiles
WOC = 512  # out-proj columns per core
SCALE = 1.0 / float(np.sqrt(HD))
ESHIFT = 12.0  # constant shift inside exp; cancels in softmax
MASKVAL = -1e30

_cache = {}


def _n_ktiles(j: int, causal: bool) -> int:
    return 4 * (j + 1) if causal else NKT


def _build(mode: str):
    """Build + compile the SPMD graph. mode in {'causal', 'none', 'general'}."""
    import concourse.bass as bass
    import concourse.mybir as mybir
    import concourse.tile as tile
    from concourse import bacc
    from concourse.masks import make_identity

    causal = mode == "causal"
    general = mode == "general"
    f32 = mybir.dt.float32
    bf16 = mybir.dt.bfloat16

    bias_dve = os.environ.get("KOPT_BIAS_DVE", "1") == "1"
    rope_gps = os.environ.get("KOPT_ROPE_GPS", "0") == "1"

    nc = bacc.Bacc("TRN2", target_bir_lowering=False, debug=False, num_devices=NCORES)

    xt_e = nc.dram_tensor("xt", [DIM, S], bf16, kind="ExternalInput")
    wq_e = nc.dram_tensor("wq", [DIM, QH * HD], bf16, kind="ExternalInput")
    wk_e = nc.dram_tensor("wk", [DIM, HD], bf16, kind="ExternalInput")
    wv_e = nc.dram_tensor("wv", [DIM, HD], bf16, kind="ExternalInput")
    woc_e = nc.dram_tensor("woc", [NH * HD, WOC], bf16, kind="ExternalInput")
    cos_e = nc.dram_tensor("cosT", [HD, S], f32, kind="ExternalInput")
    sin_e = nc.dram_tensor("sinT", [HD, S], f32, kind="ExternalInput")
    ones_e = nc.dram_tensor("ones", [HD, HD], bf16, kind="ExternalInput")
    if causal:
        biasd_e = nc.dram_tensor("biasd", [HD, 4 * SC], f32, kind="ExternalInput")
    if general:
        maskb_e = nc.dram_tensor("maskb", [S, S], f32, kind="ExternalInput")
    out_e = nc.dram_tensor("out", [S, WOC], bf16, kind="ExternalOutput")

    with tile.TileContext(nc) as tc:
        with (
            tc.tile_pool(name="res", bufs=1) as res,
            tc.tile_pool(name="wqkv", bufs=1) as wp,
            tc.tile_pool(name="xp", bufs=24) as xp,
            tc.tile_pool(name="p1t", bufs=3) as p1t,
            tc.tile_pool(name="vtp", bufs=1) as vtp,
            tc.tile_pool(name="p2", bufs=6) as p2,
            tc.tile_pool(name="sump", bufs=5) as sump,
            tc.tile_pool(name="mb", bufs=4) as mbp,
            tc.tile_pool(name="agt", bufs=2) as agp,
            tc.tile_pool(name="osb", bufs=4) as osbp,
            tc.tile_pool(name="dram", bufs=1, space="DRAM") as dram,
            tc.tile_pool(name="gp", bufs=2, space="PSUM") as gpp,
            tc.tile_pool(name="scp", bufs=4, space="PSUM") as scp,
            tc.tile_pool(name="avp", bufs=2, space="PSUM") as avp,
        ):
            # ---- resident tiles ----
            qT = [res.tile([HD, S], bf16, tag=f"qT{h}", name=f"qT{h}") for h in range(QH)]
            kT = res.tile([HD, S], bf16, tag="kT")
            V = res.tile([HD, S], bf16, tag="V")  # cols [128kc:+128] = V chunk kc
            cosT = res.tile([HD, S], f32, tag="cosT")
            sinT = res.tile([HD, S], f32, tag="sinT")
            ident = res.tile([HD, HD], f32, tag="ident")
            identr = res.tile([HD, HD], bf16, tag="identr")
            ones = res.tile([HD, HD], bf16, tag="ones")
            ebias = res.tile([128, 1], f32, tag="ebias")
            woc_t = [res.tile([128, WOC], bf16, tag=f"woc{cc}", name=f"woc{cc}") for cc in range(NKT)]
            if causal:
                biasd = res.tile([HD, 4 * SC], f32, tag="biasd")
            # phase-1 weights, resident
            wq_d = [wp.tile([128, QH * HD], bf16, tag=f"wqd{d}", name=f"wqd{d}") for d in range(NKT)]
            wk_t = [wp.tile([128, 128], bf16, tag=f"wk{d}", name=f"wk{d}") for d in range(NKT)]
            wv_t = [wp.tile([128, 128], bf16, tag=f"wv{d}", name=f"wv{d}") for d in range(NKT)]
            vT = vtp.tile([HD, S], f32, tag="vT")

            # bounce buffers for the attnT AllGathers
            bnc_in = dram.tile([4 * SC, SC], bf16)  # rows [512j:+512] = chunk j send
            bnc_out = dram.tile([4 * QH * NKV * HD, SC], bf16)  # (8192, 512)
            wup_in = dram.tile([128, 4], bf16)
            wup_out = dram.tile([512, 4], bf16)

            def warmup_ag():
                """Tiny AllGather at kernel start: pays first-collective
                setup cost and aligns the ranks while proj0 computes."""
                wt = res.tile([128, 4], bf16, tag="wup", name="wup")
                nc.vector.memset(wt[:, :], 0.0)
                nc.sync.dma_start(out=wup_in[:, :], in_=wt[:, :])
                nc.gpsimd.collective_compute(
                    "AllGather",
                    bass.mybir.AluOpType.bypass,
                    replica_groups=[[0, 1, 2, 3], [4, 5, 6, 7]],
                    ins=[wup_in[:, :].opt()],
                    outs=[wup_out[:, :].opt()],
                )

            # per-round readback tiles of the gathered attnT chunk
            agt = {}

            def prelude_dmas():
                """Everything not needed in the first ~10us, issued after the
                first s-chunk's x/wq tiles so the pipeline starts early."""
                nc.sync.dma_start(out=ones[:, :], in_=ones_e[:, :])
                if causal:
                    nc.sync.dma_start(out=biasd[:, :], in_=biasd_e[:, :])
                make_identity(nc, ident[:, :])
                nc.vector.tensor_copy(identr[:, :], ident[:, :])
                nc.vector.memset(ebias[:, :], -ESHIFT)

            def woc_dmas():
                for cc in range(NKT):
                    nc.sync.dma_start(
                        out=woc_t[cc][:, :], in_=woc_e[128 * cc : 128 * cc + 128, :]
                    )

            def rope_evict(psum, dst, sl):
                """dst[:, sl] = rotate(psum); cosT/sinT are [c;c]/[s;s]
                stacked. m2s holds the sin product with halves swapped so
                the combine steps see equal base partitions. The combines
                run on gpsimd (SBUF-only) to unload the DVE."""
                m1 = p1t.tile([128, SC], f32, tag="t1", name="m1")
                m2s = p1t.tile([128, SC], f32, tag="t2", name="m2s")
                nc.vector.tensor_mul(m1[:, :], psum[:, :], cosT[:, sl])
                nc.vector.tensor_mul(m2s[64:128, :], psum[0:64, :], sinT[0:64, sl])
                nc.vector.tensor_mul(m2s[0:64, :], psum[64:128, :], sinT[64:128, sl])
                eng = nc.gpsimd if rope_gps else nc.vector
                eng.tensor_sub(dst[0:64, sl], m1[0:64, :], m2s[0:64, :])
                eng.tensor_add(dst[64:128, sl], m1[64:128, :], m2s[64:128, :])

            def proj_sc(sc):
                """QKV projection + RoPE for s-chunk sc, plus V transposes of
                this chunk's four 128-column blocks."""
                sl = slice(SC * sc, SC * sc + SC)
                xts = [xp.tile([128, SC], bf16, tag="xp", name="xp") for _ in range(NKT)]
                for d in range(NKT):
                    nc.sync.dma_start(
                        out=xts[d][:, :], in_=xt_e[128 * d : 128 * d + 128, sl]
                    )
                    if sc == 0:
                        # wq head-column 0 first so the first matmul chain can
                        # start as early as possible
                        nc.sync.dma_start(
                            out=wq_d[d][:, 0:128], in_=wq_e[128 * d : 128 * d + 128, 0:128]
                        )
                nc.sync.dma_start(out=cosT[:, sl], in_=cos_e[:, sl])
                nc.sync.dma_start(out=sinT[:, sl], in_=sin_e[:, sl])
                if sc == 0:
                    for d in range(NKT):
                        nc.sync.dma_start(
                            out=wq_d[d][:, 128:512],
                            in_=wq_e[128 * d : 128 * d + 128, 128:512],
                        )
                        nc.sync.dma_start(
                            out=wk_t[d][:, :], in_=wk_e[128 * d : 128 * d + 128, :]
                        )
                        nc.sync.dma_start(
                            out=wv_t[d][:, :], in_=wv_e[128 * d : 128 * d + 128, :]
                        )
                    prelude_dmas()
                for h in range(QH):
                    ps = gpp.tile([128, SC], f32, tag="gp")
                    for d in range(NKT):
                        nc.tensor.matmul(
                            ps[:, :],
                            lhsT=wq_d[d][:, 128 * h : 128 * h + 128],
                            rhs=xts[d][:, :],
                            start=(d == 0),
                            stop=(d == NKT - 1),
                        )
                    rope_evict(ps, qT[h], sl)
                ps = gpp.tile([128, SC], f32, tag="gp")
                for d in range(NKT):
                    nc.tensor.matmul(
                        ps[:, :],
                        lhsT=wk_t[d][:, :],
                        rhs=xts[d][:, :],
                        start=(d == 0),
                        stop=(d == NKT - 1),
                    )
                rope_evict(ps, kT, sl)
                ps = gpp.tile([128, SC], f32, tag="gp")
                for d in range(NKT):
                    nc.tensor.matmul(
                        ps[:, :],
                        lhsT=wv_t[d][:, :],
                        rhs=xts[d][:, :],
                        start=(d == 0),
                        stop=(d == NKT - 1),
                    )
                nc.scalar.copy(vT[:, sl], ps[:, :])
                # transpose this chunk's vT columns -> V (shared score psum)
                for kc in range(4 * sc, 4 * sc + 4):
                    cs = slice(128 * kc, 128 * kc + 128)
                    pst = scp.tile([128, SC], f32, tag="sc", name="vtr")
                    nc.tensor.transpose(pst[:, 0:128], vT[:, cs], ident[:, :])
                    nc.scalar.copy(V[:, cs], pst[:, 0:128])

            def attn_round(j, hs):
                """Attention for q-chunk j, heads hs; writes at tiles to
                bnc_in rows [512j + 128h]. Exp-tile pair sums run on gpsimd
                (spread through the round) so the DVE chain stays short."""
                qsl = slice(SC * j, SC * j + SC)
                nkt = _n_ktiles(j, causal)
                for h in hs:
                    av_ps = avp.tile([HD, SC], f32, tag="av")
                    es = []  # exp tiles, then pair/quad reduced
                    for kt in range(nkt):
                        ks = slice(128 * kt, 128 * kt + 128)
                        sc_ps = scp.tile([128, SC], f32, tag="sc")
                        is_diag = causal and kt >= nkt - 4
                        use_mm_bias = (is_diag and not bias_dve) or general
                        nc.tensor.matmul(
                            sc_ps[:, :],
                            lhsT=kT[:, ks],
                            rhs=qT[h][:, qsl],
                            start=True,
                            stop=not use_mm_bias,
                        )
                        if is_diag and bias_dve:
                            di = kt - (nkt - 4)
                            nc.vector.tensor_add(
                                sc_ps[:, :], sc_ps[:, :], biasd[:, SC * di : SC * di + SC]
                            )
                        elif is_diag:
                            di = kt - (nkt - 4)
                            bb = mbp.tile([128, SC], bf16, tag="mb")
                            nc.vector.tensor_copy(bb[:, :], biasd[:, SC * di : SC * di + SC])
                            nc.tensor.matmul(
                                sc_ps[:, :],
                                lhsT=identr[:, :],
                                rhs=bb[:, :],
                                start=False,
                                stop=True,
                            )
                        elif general:
                            mbf = mbp.tile([128, SC], f32, tag="mbf")
                            nc.sync.dma_start(
                                out=mbf[:, :],
                                in_=maskb_e[128 * kt : 128 * kt + 128, qsl],
                            )
                            nc.vector.tensor_add(sc_ps[:, :], sc_ps[:, :], mbf[:, :])
                        e_sb = p2.tile([128, SC], bf16, tag="e")
                        nc.scalar.activation(
                            e_sb[:, :],
                            sc_ps[:, :],
                            mybir.ActivationFunctionType.Exp,
                            bias=ebias[:, :],
                            scale=SCALE,
                        )
                        nc.tensor.matmul(
                            av_ps[:, :],
                            lhsT=V[:, ks],
                            rhs=e_sb[:, :],
                            start=(kt == 0),
                            stop=(kt == nkt - 1),
                        )
                        es.append(e_sb)
                        if kt % 2 == 1:
                            ep = sump.tile([128, SC], bf16, tag="ep", name="ep")
                            nc.vector.tensor_add(ep[:, :], es[-2][:, :], es[-1][:, :])
                            es[-2:] = [ep]
                            if kt % 4 == 3:
                                eq = sump.tile([128, SC], bf16, tag="eq", name="eq")
                                nc.vector.tensor_add(eq[:, :], es[-2][:, :], es[-1][:, :])
                                es[-2:] = [eq]
                    # es now holds nkt/4 quad tiles; sum over k via ones-matmul
                    sum_ps = scp.tile([128, SC], f32, tag="sc", name="sums")
                    for qi, eq in enumerate(es):
                        nc.tensor.matmul(
                            sum_ps[:, :],
                            lhsT=ones[:, :],
                            rhs=eq[:, :],
                            start=(qi == 0),
                            stop=(qi == len(es) - 1),
                        )
                    rec = p2.tile([128, SC], f32, tag="rec")
                    nc.vector.reciprocal_approx_fast(rec[:, :], sum_ps[:, :])
                    at = p2.tile([HD, SC], bf16, tag="at")
                    nc.vector.tensor_mul(at[:, :], av_ps[:, :], rec[:, :])
                    nc.scalar.dma_start(
                        out=bnc_in[SC * j + HD * h : SC * j + HD * h + HD, :],
                        in_=at[:, :],
                    )

            def ag_fire(j, half=None):
                """AllGather bnc_in chunk j (or a head-pair half of it) to
                bnc_out. half=0: heads {0,1} of every rank; half=1: {2,3}."""
                if half is None:
                    isl = slice(SC * j, SC * j + SC)
                    osl = slice(2048 * j, 2048 * j + 2048)
                else:
                    isl = slice(SC * j + 256 * half, SC * j + 256 * half + 256)
                    osl = slice(2048 * j + 1024 * half, 2048 * j + 1024 * half + 1024)
                if os.environ.get("KOPT_NOCC", "0") == "1":
                    nrows = isl.stop - isl.start
                    nc.sync.dma_start(
                        out=bnc_out[osl.start : osl.start + nrows, :],
                        in_=bnc_in[isl, :],
                    )
                else:
                    nc.gpsimd.collective_compute(
                        "AllGather",
                        bass.mybir.AluOpType.bypass,
                        replica_groups=[[0, 1, 2, 3], [4, 5, 6, 7]],
                        ins=[bnc_in[isl, :].opt()],
                        outs=[bnc_out[osl, :].opt()],
                    )

            def readback(j, half=None):
                """Gathered rows -> SBUF contraction tiles. Emitted just
                before their out-projection so the DMA lanes never stall on
                an in-flight AllGather. Rank blocks are ordered by rank =
                head-group, so m-chunk cc sits at a static offset."""
                if half is None:
                    ccs = list(range(NKT))
                else:
                    ccs = [r * 4 + 2 * half + i for r in range(4) for i in range(2)]
                for i, cc in enumerate(ccs):
                    t = agp.tile([128, SC], bf16, tag=f"ag{cc}", name=f"ag{cc}")
                    if half is None:
                        src = bnc_out[2048 * j + 128 * cc : 2048 * j + 128 * cc + 128, :]
                    else:
                        base = 2048 * j + 1024 * half
                        r, k = divmod(i, 2)
                        src = bnc_out[base + 256 * r + 128 * k : base + 256 * r + 128 * k + 128, :]
                    nc.sync.dma_start(out=t[:, :], in_=src)
                    agt[(j, cc)] = t

            def outproj_pair(j, sts, ccs, start=True, stop=True, pso=None):
                """Accumulate out rows [512j + 128st] (this core's 512
                columns) for the two q-row blocks in sts, contracting over
                m-chunks ccs of the gathered attnT chunk j. With stop=False
                returns the live psum pair for a later call."""
                if pso is None:
                    pso = [gpp.tile([128, WOC], f32, tag="gp", name="op") for _ in range(2)]
                for sti, st in enumerate(sts):
                    for ci, cc in enumerate(ccs):
                        nc.tensor.matmul(
                            pso[sti][:, :],
                            lhsT=agt[(j, cc)][:, 128 * st : 128 * st + 128],
                            rhs=woc_t[cc][:, :],
                            start=(start and ci == 0),
                            stop=(stop and ci == len(ccs) - 1),
                        )
                if stop:
                    for sti, st in enumerate(sts):
                        ob = osbp.tile([128, WOC], bf16, tag="ob", name="ob")
                        if sti % 2 == 0:
                            nc.scalar.copy(ob[:, :], pso[sti][:, :])
                        else:
                            nc.vector.tensor_copy(ob[:, :], pso[sti][:, :])
                        nc.sync.dma_start(
                            out=out_e[SC * j + 128 * st : SC * j + 128 * st + 128, :],
                            in_=ob[:, :],
                        )
                return pso

            def outproj(j):
                outproj_pair(j, [0, 1], list(range(NKT)))
                outproj_pair(j, [2, 3], list(range(NKT)))

            # ---- schedule ----
            if os.environ.get("KOPT_WARMUP_AG", "1") == "1":
                warmup_ag()
            if causal:
                proj_sc(0)
                attn_round(0, range(QH))
                ag_fire(0)
                proj_sc(1)
                woc_dmas()
                attn_round(1, range(QH))
                ag_fire(1)
                proj_sc(2)
                attn_round(2, range(QH))
                ag_fire(2)
                readback(0)
                outproj(0)
                proj_sc(3)
                attn_round(3, [0, 1])
                ag_fire(3, half=0)
                readback(1)
                outproj(1)
                attn_round(3, [2, 3])
                ag_fire(3, half=1)
                readback(2)
                outproj(2)
            else:
                for sc in range(NSC):
                    proj_sc(sc)
                woc_dmas()
                attn_round(0, range(QH))
                ag_fire(0)
                attn_round(1, range(QH))
                ag_fire(1)
                readback(0)
                outproj(0)
                attn_round(2, range(QH))
                ag_fire(2)
                readback(1)
                outproj(1)
                attn_round(3, [0, 1])
                ag_fire(3, half=0)
                attn_round(3, [2, 3])
                ag_fire(3, half=1)
                readback(2)
                outproj(2)
            half0 = [r * 4 + i for r in range(4) for i in range(2)]
            half1 = [r * 4 + 2 + i for r in range(4) for i in range(2)]
            readback(3, half=0)
            psoA = outproj_pair(3, [0, 1], half0, start=True, stop=False)
            readback(3, half=1)
            outproj_pair(3, [0, 1], half1, start=False, stop=True, pso=psoA)
            psoB = outproj_pair(3, [2, 3], half0, start=True, stop=False)
            outproj_pair(3, [2, 3], half1, start=False, stop=True, pso=psoB)

    nc.compile()
    return nc


def _perm_cols(w: np.ndarray, heads: list) -> np.ndarray:
    """Reorder head columns to [even dims; odd dims] for block RoPE."""
    cols = []
    for h in heads:
        base = HD * h
        cols.extend([base + 2 * i for i in range(HD // 2)])
        cols.extend([base + 2 * i + 1 for i in range(HD // 2)])
    return np.ascontiguousarray(w[:, cols])


def kernel(x, wq, wk, wv, wo, freqs_cos, freqs_sin, mask):
    from concourse.bass_utils import run_bass_kernel_spmd

    x = np.asarray(x, dtype=np.float32)
    wq = np.asarray(wq, dtype=np.float32)
    wk = np.asarray(wk, dtype=np.float32)
    wv = np.asarray(wv, dtype=np.float32)
    wo = np.asarray(wo, dtype=np.float32)
    freqs_cos = np.asarray(freqs_cos, dtype=np.float32)
    freqs_sin = np.asarray(freqs_sin, dtype=np.float32)
    mask = np.asarray(mask)

    if not mask.any():
        mode = "none"
    elif np.array_equal(mask, np.triu(np.ones((S, S), dtype=bool), k=1)):
        mode = "causal"
    else:
        mode = "general"

    if mode not in _cache:
        import time as _t

        t0 = _t.time()
        _cache[mode] = _build(mode)
        print(f"[kernel] built mode={mode} in {_t.time() - t0:.1f}s", flush=True)
    nc = _cache[mode]

    # ---- host-side prep (sharding + layout) ----
    import ml_dtypes

    xt = [np.ascontiguousarray(x[b].T).astype(ml_dtypes.bfloat16) for b in range(B)]
    wo_r = wo.astype(ml_dtypes.bfloat16)
    cosT = np.ascontiguousarray(np.concatenate([freqs_cos.T, freqs_cos.T], axis=0))
    sinT = np.ascontiguousarray(np.concatenate([freqs_sin.T, freqs_sin.T], axis=0))
    ones = np.ones((HD, HD), dtype=ml_dtypes.bfloat16)

    if mode == "causal":
        # 4 diag patterns (delta = 0,128,256,384) packed as (128, 2048):
        # bias[i, 512*di + jq] = MASKVAL if (128*di + i) > jq else 0
        i_ = np.arange(HD)[:, None]
        jq = np.arange(SC)[None, :]
        biasd = np.concatenate(
            [
                np.where(128 * di + i_ > jq, np.float32(MASKVAL), np.float32(0.0))
                for di in range(4)
            ],
            axis=1,
        ).astype(np.float32)
    if mode == "general":
        maskb = np.ascontiguousarray(
            np.where(mask.T, np.float32(MASKVAL), np.float32(0.0))
        ).astype(np.float32)

    in_maps = []
    for core in range(NCORES):
        b, g = divmod(core, 4)
        heads = [QH * g + h for h in range(QH)]
        m = {
            "xt": xt[b],
            "wq": _perm_cols(wq, heads).astype(ml_dtypes.bfloat16),
            "wk": _perm_cols(wk, [g]).astype(ml_dtypes.bfloat16),
            "wv": np.ascontiguousarray(wv[:, HD * g : HD * g + HD]).astype(ml_dtypes.bfloat16),
            "woc": np.ascontiguousarray(wo_r[:, WOC * g : WOC * g + WOC]),
            "cosT": cosT,
            "sinT": sinT,
            "ones": ones,
        }
        if mode == "causal":
            m["biasd"] = biasd
        if mode == "general":
            m["maskb"] = maskb
        in_maps.append(m)

    import time as _t

    t0 = _t.time()
    print("[kernel] launching SPMD run", flush=True)
    res = run_bass_kernel_spmd(nc, in_maps, core_ids=list(range(NCORES)))
    print(f"[kernel] SPMD run done in {_t.time() - t0:.1f}s", flush=True)
    kernel._last_result = res

    out = np.empty((B, S, DIM), dtype=np.float32)
    for core in range(NCORES):
        b, g = divmod(core, 4)
        out[b, :, WOC * g : WOC * g + WOC] = res.results[core]["out"].astype(np.float32)
    return out

